# revision 25
# baseline (speedup 1.0000x reference)
"""Multi-head causal self-attention (B=1, S=4096, D=1024, H=16) on 8 TRN2 cores.

Sharding: 2 heads per core (head/tensor parallel). Each core computes its
heads' Q/K/V projections, causal flash attention, and a partial output
projection against its 128 columns of Wo. The host sums the 8 partials and
adds the output bias.

Device layouts (per core, bf16 compute):
  - x is fed transposed:  xT [D=1024, S=4096]   (model dim on partitions)
  - Q^T, K^T [128, 4096]: per-core head dims on partitions (h0: 0-63, h1: 64-127)
  - V natural [4096, 130]: per seq-tile [128, 65*2] = [V_h0 | ones | V_h1 | ones]
    The ones column makes the PV matmul also produce the softmax denominator.
  - scores are computed transposed S^T[k, q] so the PV matmul needs no
    transposition; softmax is exp-only (scores are bounded, no max-subtract).
  - output is written transposed outT [1024, 4096] fp32 (partial; host sums).
"""

import numpy as np
import ml_dtypes
from contextlib import ExitStack

import concourse.bass as bass
import concourse.tile as tile
from concourse import bacc, mybir
from concourse.bass_utils import run_bass_kernel_spmd

P = 128
S = 4096
D = 1024
DH = 64
N_CORES = 8
SCALE = 1.0 / 8.0  # 1/sqrt(64)
NQ = 512           # query block (matmul free dim)
KT = 128           # key tile (contraction partitions)
NQB = S // NQ      # 8 query blocks
NKT = S // KT      # 32 key tiles
KO = D // P        # 8 contraction subtiles over the model dim

BF16 = mybir.dt.bfloat16
F32 = mybir.dt.float32
EXP = mybir.ActivationFunctionType.Exp
ADD = mybir.AluOpType.add


def _emit(tc, xT, wqT, wkT, wvT, woT, bqk, masks, outT, dbg=None):
    nc = tc.nc
    with ExitStack() as ctx:
        from collections import deque
        from concourse.masks import make_identity

        const = ctx.enter_context(tc.tile_pool(name="const", bufs=1))

        xT_r = xT.rearrange("(ko p) n -> p ko n", p=P)
        xT_sb = const.tile([P, KO, S], BF16)
        for n in range(NQB):  # chunked so projections can start early
            nsl = slice(n * NQ, (n + 1) * NQ)
            nc.sync.dma_start(xT_sb[:, :, nsl], xT_r[:, :, nsl])
        wq_sb = const.tile([P, KO, P], BF16)
        nc.sync.dma_start(wq_sb, wqT.rearrange("(ko p) m -> p ko m", p=P))
        wk_sb = const.tile([P, KO, P], BF16)
        nc.sync.dma_start(wk_sb, wkT.rearrange("(ko p) m -> p ko m", p=P))
        wv_sb = const.tile([P, KO, P], BF16)
        nc.sync.dma_start(wv_sb, wvT.rearrange("(ko p) m -> p ko m", p=P))
        wo_sb = const.tile([P, D], BF16)
        nc.sync.dma_start(wo_sb, woT)
        bqk_sb = const.tile([P, 3], F32)
        nc.sync.dma_start(bqk_sb, bqk.rearrange("b p -> p b"))
        masks_sb = const.tile([P, 4, NQ], BF16)
        nc.sync.dma_start(masks_sb, masks)

        qT_sb = const.tile([P, S], BF16)
        kT_sb = const.tile([P, S], BF16)
        vT_sb = const.tile([P, S], BF16)
        v_sb = const.tile([P, S // P, 130], BF16)
        attnT_sb = const.tile([P, S], BF16)
        nc.vector.memset(v_sb, 1.0)  # presets the two ones-columns

        ident = const.tile([P, P], BF16)
        make_identity(nc, ident)

        # PSUM budget (8 banks): ppool 2 (proj accum + V transpose), spool 4
        # (two [128,1024] score slabs; oproj tiles borrow these slots), vpool 2.
        ppool = ctx.enter_context(tc.tile_pool(name="proj_psum", bufs=2, space="PSUM"))
        spool = ctx.enter_context(tc.tile_pool(name="score_psum", bufs=2, space="PSUM"))
        vpool = ctx.enter_context(tc.tile_pool(name="pv_psum", bufs=1, space="PSUM"))
        work = ctx.enter_context(tc.tile_pool(name="work", bufs=4))
        nwork = ctx.enter_context(tc.tile_pool(name="nwork", bufs=3))
        dpool = ctx.enter_context(tc.tile_pool(name="dscratch", bufs=2, space="DRAM"))

        def proj_chunk(bcol, w_sb, dst, n):
            def emit():
                ps = ppool.tile([P, NQ], F32, tag="ps", name=f"ps_{bcol}_{n}")
                for kt in range(KO):
                    nc.tensor.matmul(
                        ps,
                        lhsT=w_sb[:, kt, :],
                        rhs=xT_sb[:, kt, n * NQ:(n + 1) * NQ],
                        start=(kt == 0),
                        stop=(kt == KO - 1),
                    )
                nc.vector.tensor_tensor(
                    dst[:, n * NQ:(n + 1) * NQ],
                    ps,
                    bqk_sb[:, bcol:bcol + 1].to_broadcast([P, NQ]),
                    op=ADD,
                )
            return emit

        def v_transpose(t):
            def emit():
                tp = ppool.tile([P, P], BF16, tag="ps", name=f"tp_{t}")
                nc.tensor.transpose(tp, vT_sb[:, t * P:(t + 1) * P], ident)
                nc.vector.tensor_copy(
                    v_sb[:, t, :].rearrange("p (h x) -> p h x", x=65)[:, :, 0:DH],
                    tp.rearrange("p (h x) -> p h x", x=DH),
                )
            return emit

        def proj_ops(nb):
            ops = [
                proj_chunk(0, wq_sb, qT_sb, nb),
                proj_chunk(1, wk_sb, kT_sb, nb),
                proj_chunk(2, wv_sb, vT_sb, nb),
            ]
            ops += [v_transpose(t) for t in range(4 * nb, 4 * nb + 4)]
            return deque(ops)

        def emit_oproj(b):
            qsl = slice(b * NQ, (b + 1) * NQ)
            for m in range(D // P):
                po = spool.tile([P, NQ], F32, tag="slab", name=f"po_{b}_{m}")
                nc.tensor.matmul(
                    po,
                    lhsT=wo_sb[:, m * P:(m + 1) * P],
                    rhs=attnT_sb[:, qsl],
                    start=True,
                    stop=True,
                )
                ot = work.tile([P, NQ], F32, tag="ot", name=f"ot_{b}_{m}")
                nc.vector.tensor_copy(ot, po)
                nc.gpsimd.dma_start(
                    outT.rearrange("(mo p) n -> p mo n", p=P)[:, m, qsl], ot
                )

        # block 0's projections up front
        for op in proj_ops(0):
            op()

        for b in range(NQB):
            nk = 4 * (b + 1)  # causal: only key tiles up to the diagonal
            pending = proj_ops(b + 1) if b + 1 < NQB else deque()
            pvs = [
                vpool.tile([DH + 1, NQ], F32, tag=f"pv{h}", name=f"pv{h}_{b}")
                for h in (0, 1)
            ]
            qsl = slice(b * NQ, (b + 1) * NQ)
            for kt in range(nk):
                slab = spool.tile([P, 2 * NQ], F32, tag="slab")
                for h in (0, 1):
                    nc.tensor.matmul(
                        slab[:, h * NQ:(h + 1) * NQ],
                        lhsT=kT_sb[h * DH:(h + 1) * DH, kt * KT:(kt + 1) * KT],
                        rhs=qT_sb[h * DH:(h + 1) * DH, qsl],
                        start=True,
                        stop=True,
                    )
                pT = work.tile([P, 2 * NQ], BF16, tag="pT")
                nc.scalar.activation(pT, slab, EXP, scale=SCALE)
                if kt >= 4 * b:
                    j = kt - 4 * b
                    for h in (0, 1):
                        nc.vector.tensor_mul(
                            pT[:, h * NQ:(h + 1) * NQ],
                            pT[:, h * NQ:(h + 1) * NQ],
                            masks_sb[:, j, :],
                        )
                for h in (0, 1):
                    nc.tensor.matmul(
                        pvs[h],
                        lhsT=v_sb[:, kt, h * 65:(h + 1) * 65],
                        rhs=pT[:, h * NQ:(h + 1) * NQ],
                        start=(kt == 0),
                        stop=(kt == nk - 1),
                    )
                if pending:
                    pending.popleft()()  # next block's projections, spread out
                if kt == 3 and b > 0:
                    # previous block's output projection, emitted here so its
                    # PE work lands while this block's attention is in flight
                    emit_oproj(b - 1)
            while pending:
                pending.popleft()()
            for h in (0, 1):
                pvS = nwork.tile([DH + 1, NQ], F32, tag="pvS")
                nc.vector.tensor_copy(pvS, pvs[h])  # frees the PSUM slot early
                rcp0 = nwork.tile([1, NQ], F32, tag="rcp0")
                nc.vector.tensor_copy(rcp0, pvS[DH:DH + 1, :])  # sums to part 0
                nc.vector.reciprocal_approx_fast(rcp0, rcp0)
                scr = dpool.tile([NQ], F32, tag="scr")
                nc.sync.dma_start(scr, rcp0)
                rb = nwork.tile([DH, NQ], F32, tag="rb")
                nc.sync.dma_start(rb, scr[None, :].to_broadcast([DH, NQ]))
                tmp = nwork.tile([DH, NQ], BF16, tag="tmp")
                nc.vector.tensor_mul(tmp, pvS[0:DH, :], rb)
                nc.sync.dma_start(attnT_sb[h * DH:(h + 1) * DH, qsl], tmp)
                if dbg is not None and b == 1:
                    nc.sync.dma_start(dbg[f"pv{h}"], pvS)
        emit_oproj(NQB - 1)
        if dbg is not None:
            nc.sync.dma_start(dbg["qT"], qT_sb)
            nc.sync.dma_start(dbg["kT"], kT_sb)
            nc.sync.dma_start(dbg["v"], v_sb)
            nc.sync.dma_start(dbg["attnT"], attnT_sb)


def build(debug_out=False):
    nc = bacc.Bacc(
        "TRN2",
        target_bir_lowering=False,
        debug=False,
        enable_asserts=False,
    )
    xT = nc.dram_tensor("xT", [D, S], BF16, kind="ExternalInput").ap()
    wqT = nc.dram_tensor("wqT", [D, P], BF16, kind="ExternalInput").ap()
    wkT = nc.dram_tensor("wkT", [D, P], BF16, kind="ExternalInput").ap()
    wvT = nc.dram_tensor("wvT", [D, P], BF16, kind="ExternalInput").ap()
    woT = nc.dram_tensor("woT", [P, D], BF16, kind="ExternalInput").ap()
    bqk = nc.dram_tensor("bqk", [3, P], F32, kind="ExternalInput").ap()
    masks = nc.dram_tensor("masks", [P, 4, NQ], BF16, kind="ExternalInput").ap()
    outT = nc.dram_tensor("outT", [D, S], F32, kind="ExternalOutput").ap()
    dbg = None
    if debug_out:
        dbg = {
            "qT": nc.dram_tensor("dbg_qT", [P, S], BF16, kind="ExternalOutput").ap(),
            "kT": nc.dram_tensor("dbg_kT", [P, S], BF16, kind="ExternalOutput").ap(),
            "v": nc.dram_tensor("dbg_v", [P, S // P, 130], BF16, kind="ExternalOutput").ap(),
            "attnT": nc.dram_tensor("dbg_attnT", [P, S], BF16, kind="ExternalOutput").ap(),
            "pv0": nc.dram_tensor("dbg_pv0", [DH + 1, NQ], F32, kind="ExternalOutput").ap(),
            "pv1": nc.dram_tensor("dbg_pv1", [DH + 1, NQ], F32, kind="ExternalOutput").ap(),
        }

    with tile.TileContext(nc) as tc:
        _emit(tc, xT, wqT, wkT, wvT, woT, bqk, masks, outT, dbg=dbg)
    nc.compile()
    return nc


def _make_masks():
    k = np.arange(P)[:, None]
    q = np.arange(NQ)[None, :]
    m = np.zeros((P, 4, NQ), np.float32)
    for j in range(4):
        m[:, j, :] = ((KT * j + k) <= q).astype(np.float32)
    return m.astype(ml_dtypes.bfloat16)


_STATE = {}


def _prep_inputs(x, Wq, bq, Wk, bk, Wv, bv, Wo, bo):
    bf = ml_dtypes.bfloat16
    xT = np.ascontiguousarray(np.asarray(x, np.float32).reshape(S, D).T).astype(bf)
    masks = _make_masks()
    Wq = np.asarray(Wq, np.float32)
    Wk = np.asarray(Wk, np.float32)
    Wv = np.asarray(Wv, np.float32)
    Wo = np.asarray(Wo, np.float32)
    bq = np.asarray(bq, np.float32)
    bk = np.asarray(bk, np.float32)
    bv = np.asarray(bv, np.float32)
    in_maps = []
    for c in range(N_CORES):
        r = slice(c * P, (c + 1) * P)
        in_maps.append({
            "xT": xT,
            "wqT": np.ascontiguousarray(Wq[r].T).astype(bf),
            "wkT": np.ascontiguousarray(Wk[r].T).astype(bf),
            "wvT": np.ascontiguousarray(Wv[r].T).astype(bf),
            "woT": np.ascontiguousarray(Wo[:, r].T).astype(bf),
            "bqk": np.stack([bq[r], bk[r], bv[r]]),
            "masks": masks,
        })
    return in_maps


def kernel(x, Wq, bq, Wk, bk, Wv, bv, Wo, bo):
    if "nc" not in _STATE:
        _STATE["nc"] = build()
    nc = _STATE["nc"]
    in_maps = _prep_inputs(x, Wq, bq, Wk, bk, Wv, bv, Wo, bo)
    res = run_bass_kernel_spmd(nc, in_maps, core_ids=list(range(N_CORES)))
    total = res.results[0]["outT"].astype(np.float32, copy=True)
    for c in range(1, N_CORES):
        total += res.results[c]["outT"]
    out = total.T + np.asarray(bo, np.float32)[None, :]
    return np.ascontiguousarray(out, dtype=np.float32).reshape(1, S, D)


# revision 28
# speedup vs baseline: 1.2914x; 1.2914x over previous
"""Multi-head causal self-attention (B=1, S=4096, D=1024, H=16) on 8 TRN2 cores.

Sharding: 2 heads per core (head/tensor parallel). Each core computes its
heads' Q/K/V projections, causal flash attention, and a partial output
projection against its 128 columns of Wo. The host sums the 8 partials and
adds the output bias.

Device layouts (per core, bf16 compute):
  - x is fed transposed:  xT [D=1024, S=4096]   (model dim on partitions)
  - Q^T, K^T [128, 4096]: per-core head dims on partitions (h0: 0-63, h1: 64-127)
  - V natural [4096, 130]: per seq-tile [128, 65*2] = [V_h0 | ones | V_h1 | ones]
    The ones column makes the PV matmul also produce the softmax denominator.
  - scores are computed transposed S^T[k, q] so the PV matmul needs no
    transposition; softmax is exp-only (scores are bounded, no max-subtract).
  - output is written transposed outT [1024, 4096] fp32 (partial; host sums).
"""

import numpy as np
import ml_dtypes
from contextlib import ExitStack

import concourse.bass as bass
import concourse.tile as tile
from concourse import bacc, mybir
from concourse.bass_utils import run_bass_kernel_spmd

P = 128
S = 4096
D = 1024
DH = 64
N_CORES = 8
SCALE = 1.0 / 8.0  # 1/sqrt(64)
NQ = 512           # query block (matmul free dim)
KT = 128           # key tile (contraction partitions)
NQB = S // NQ      # 8 query blocks
NKT = S // KT      # 32 key tiles
KO = D // P        # 8 contraction subtiles over the model dim

BF16 = mybir.dt.bfloat16
F32 = mybir.dt.float32
EXP = mybir.ActivationFunctionType.Exp
ADD = mybir.AluOpType.add


def _emit(tc, xT, wqT, wkT, wvT, woT, bqk, masks, outT, dbg=None):
    nc = tc.nc
    with ExitStack() as ctx:
        from collections import deque
        from concourse.masks import make_identity

        const = ctx.enter_context(tc.tile_pool(name="const", bufs=1))

        xT_r = xT.rearrange("(ko p) n -> p ko n", p=P)
        xT_sb = const.tile([P, KO, S], BF16)
        for n in range(NQB):  # chunked so projections can start early
            nsl = slice(n * NQ, (n + 1) * NQ)
            nc.sync.dma_start(xT_sb[:, :, nsl], xT_r[:, :, nsl])
        wq_sb = const.tile([P, KO, P], BF16)
        nc.sync.dma_start(wq_sb, wqT.rearrange("(ko p) m -> p ko m", p=P))
        wk_sb = const.tile([P, KO, P], BF16)
        nc.sync.dma_start(wk_sb, wkT.rearrange("(ko p) m -> p ko m", p=P))
        wv_sb = const.tile([P, KO, P], BF16)
        nc.sync.dma_start(wv_sb, wvT.rearrange("(ko p) m -> p ko m", p=P))
        wo_sb = const.tile([P, D], BF16)
        nc.sync.dma_start(wo_sb, woT)
        bqk_sb = const.tile([P, 3], F32)
        nc.sync.dma_start(bqk_sb, bqk.rearrange("b p -> p b"))
        masks_sb = const.tile([P, 4, NQ], BF16)
        nc.sync.dma_start(masks_sb, masks)

        qT_sb = const.tile([P, S], BF16)
        kT_sb = const.tile([P, S], BF16)
        vT_sb = const.tile([P, S], BF16)
        v_sb = const.tile([P, S // P, 130], BF16)
        attnT_sb = const.tile([P, S], BF16)
        nc.vector.memset(v_sb, 1.0)  # presets the two ones-columns

        ident = const.tile([P, P], BF16)
        make_identity(nc, ident)

        # PSUM budget (8 banks): spool 4 (two [128,1024] score slabs),
        # vpool 2 (pv0/pv1 accumulators), ppool 1 (proj accum / V transpose),
        # opool 1 (output projection).
        spool = ctx.enter_context(tc.tile_pool(name="score_psum", bufs=2, space="PSUM"))
        vpool = ctx.enter_context(tc.tile_pool(name="pv_psum", bufs=1, space="PSUM"))
        ppool = ctx.enter_context(tc.tile_pool(name="proj_psum", bufs=1, space="PSUM"))
        opool = ctx.enter_context(tc.tile_pool(name="oproj_psum", bufs=1, space="PSUM"))
        work = ctx.enter_context(tc.tile_pool(name="work", bufs=4))
        nwork = ctx.enter_context(tc.tile_pool(name="nwork", bufs=3))
        dpool = ctx.enter_context(tc.tile_pool(name="dscratch", bufs=2, space="DRAM"))

        def proj_chunk(bcol, w_sb, dst, n):
            """Two pacing items of 4 accumulation matmuls each (shared psum)."""
            state = {}

            def emit_lo():
                ps = ppool.tile([P, NQ], F32, tag="ps", name=f"ps_{bcol}_{n}")
                state["ps"] = ps
                for kt in range(KO // 2):
                    nc.tensor.matmul(
                        ps,
                        lhsT=w_sb[:, kt, :],
                        rhs=xT_sb[:, kt, n * NQ:(n + 1) * NQ],
                        start=(kt == 0),
                        stop=False,
                    )

            def emit_hi():
                ps = state["ps"]
                for kt in range(KO // 2, KO):
                    nc.tensor.matmul(
                        ps,
                        lhsT=w_sb[:, kt, :],
                        rhs=xT_sb[:, kt, n * NQ:(n + 1) * NQ],
                        start=False,
                        stop=(kt == KO - 1),
                    )
                nc.vector.tensor_tensor(
                    dst[:, n * NQ:(n + 1) * NQ],
                    ps,
                    bqk_sb[:, bcol:bcol + 1].to_broadcast([P, NQ]),
                    op=ADD,
                )

            return [emit_lo, emit_hi]

        def v_transpose(t):
            def emit():
                tp = ppool.tile([P, P], BF16, tag="ps", name=f"tp_{t}")
                nc.tensor.transpose(tp, vT_sb[:, t * P:(t + 1) * P], ident)
                nc.vector.tensor_copy(
                    v_sb[:, t, :].rearrange("p (h x) -> p h x", x=65)[:, :, 0:DH],
                    tp.rearrange("p (h x) -> p h x", x=DH),
                )
            return emit

        def proj_ops(nb):
            ops = []
            ops += proj_chunk(0, wq_sb, qT_sb, nb)
            ops += proj_chunk(1, wk_sb, kT_sb, nb)
            ops += proj_chunk(2, wv_sb, vT_sb, nb)
            ops += [v_transpose(t) for t in range(4 * nb, 4 * nb + 4)]
            return ops

        def oproj_mtile(b, m):
            def emit():
                qsl = slice(b * NQ, (b + 1) * NQ)
                po = opool.tile([P, NQ], F32, tag="po", name=f"po_{b}_{m}")
                nc.tensor.matmul(
                    po,
                    lhsT=wo_sb[:, m * P:(m + 1) * P],
                    rhs=attnT_sb[:, qsl],
                    start=True,
                    stop=True,
                )
                ot = work.tile([P, NQ], F32, tag="ot", name=f"ot_{b}_{m}")
                nc.vector.tensor_copy(ot, po)
                nc.gpsimd.dma_start(
                    outT.rearrange("(mo p) n -> p mo n", p=P)[:, m, qsl], ot
                )
            return emit

        # block 0's projections up front
        for op in proj_ops(0):
            op()

        for b in range(NQB):
            nk = 4 * (b + 1)  # causal: only key tiles up to the diagonal
            pending = deque()
            if b > 0:  # previous block's output projection, paced into this one
                pending.extend(oproj_mtile(b - 1, m) for m in range(D // P))
            if b + 1 < NQB:  # next block's projections, paced into this one
                pending.extend(proj_ops(b + 1))
            pvs = [
                vpool.tile([DH + 1, NQ], F32, tag=f"pv{h}", name=f"pv{h}_{b}")
                for h in (0, 1)
            ]
            for kt in range(nk):
                j = kt - 4 * b  # >= 0 on causal-diagonal key tiles
                # on diagonal tiles only queries >= 128j can attend this tile
                q0 = max(0, j) * KT
                nq = NQ - q0
                qs0 = b * NQ + q0
                slab = spool.tile([P, 2, NQ], F32, tag="slab")
                for h in (0, 1):
                    nc.tensor.matmul(
                        slab[:, h, :nq],
                        lhsT=kT_sb[h * DH:(h + 1) * DH, kt * KT:(kt + 1) * KT],
                        rhs=qT_sb[h * DH:(h + 1) * DH, qs0:qs0 + nq],
                        start=True,
                        stop=True,
                    )
                pT = work.tile([P, 2, NQ], BF16, tag="pT")
                nc.scalar.activation(pT[:, :, :nq], slab[:, :, :nq], EXP, scale=SCALE)
                if j >= 0:
                    for h in (0, 1):
                        nc.vector.tensor_mul(
                            pT[:, h, :nq],
                            pT[:, h, :nq],
                            masks_sb[:, j, q0:],
                        )
                for h in (0, 1):
                    nc.tensor.matmul(
                        pvs[h][:, q0:],
                        lhsT=v_sb[:, kt, h * 65:(h + 1) * 65],
                        rhs=pT[:, h, :nq],
                        start=(kt == 0),
                        stop=(kt == nk - 1),
                    )
                if pending:
                    pending.popleft()()
            while pending:
                pending.popleft()()
            qsl = slice(b * NQ, (b + 1) * NQ)
            for h in (0, 1):
                pvS = nwork.tile([DH + 1, NQ], F32, tag="pvS")
                nc.vector.tensor_copy(pvS, pvs[h])  # frees the PSUM slot early
                rcp0 = nwork.tile([1, NQ], F32, tag="rcp0")
                nc.vector.tensor_copy(rcp0, pvS[DH:DH + 1, :])  # sums to part 0
                nc.vector.reciprocal_approx_fast(rcp0, rcp0)
                scr = dpool.tile([NQ], F32, tag="scr")
                nc.sync.dma_start(scr, rcp0)
                rb = nwork.tile([DH, NQ], F32, tag="rb")
                nc.sync.dma_start(rb, scr[None, :].to_broadcast([DH, NQ]))
                tmp = nwork.tile([DH, NQ], BF16, tag="tmp")
                nc.vector.tensor_mul(tmp, pvS[0:DH, :], rb)
                nc.sync.dma_start(attnT_sb[h * DH:(h + 1) * DH, qsl], tmp)
                if dbg is not None and b == 1:
                    nc.sync.dma_start(dbg[f"pv{h}"], pvS)
        for m in range(D // P):
            oproj_mtile(NQB - 1, m)()
        if dbg is not None:
            nc.sync.dma_start(dbg["qT"], qT_sb)
            nc.sync.dma_start(dbg["kT"], kT_sb)
            nc.sync.dma_start(dbg["v"], v_sb)
            nc.sync.dma_start(dbg["attnT"], attnT_sb)


def build(debug_out=False):
    nc = bacc.Bacc(
        "TRN2",
        target_bir_lowering=False,
        debug=False,
        enable_asserts=False,
    )
    xT = nc.dram_tensor("xT", [D, S], BF16, kind="ExternalInput").ap()
    wqT = nc.dram_tensor("wqT", [D, P], BF16, kind="ExternalInput").ap()
    wkT = nc.dram_tensor("wkT", [D, P], BF16, kind="ExternalInput").ap()
    wvT = nc.dram_tensor("wvT", [D, P], BF16, kind="ExternalInput").ap()
    woT = nc.dram_tensor("woT", [P, D], BF16, kind="ExternalInput").ap()
    bqk = nc.dram_tensor("bqk", [3, P], F32, kind="ExternalInput").ap()
    masks = nc.dram_tensor("masks", [P, 4, NQ], BF16, kind="ExternalInput").ap()
    outT = nc.dram_tensor("outT", [D, S], F32, kind="ExternalOutput").ap()
    dbg = None
    if debug_out:
        dbg = {
            "qT": nc.dram_tensor("dbg_qT", [P, S], BF16, kind="ExternalOutput").ap(),
            "kT": nc.dram_tensor("dbg_kT", [P, S], BF16, kind="ExternalOutput").ap(),
            "v": nc.dram_tensor("dbg_v", [P, S // P, 130], BF16, kind="ExternalOutput").ap(),
            "attnT": nc.dram_tensor("dbg_attnT", [P, S], BF16, kind="ExternalOutput").ap(),
            "pv0": nc.dram_tensor("dbg_pv0", [DH + 1, NQ], F32, kind="ExternalOutput").ap(),
            "pv1": nc.dram_tensor("dbg_pv1", [DH + 1, NQ], F32, kind="ExternalOutput").ap(),
        }

    with tile.TileContext(nc) as tc:
        _emit(tc, xT, wqT, wkT, wvT, woT, bqk, masks, outT, dbg=dbg)
    nc.compile()
    return nc


def _make_masks():
    k = np.arange(P)[:, None]
    q = np.arange(NQ)[None, :]
    m = np.zeros((P, 4, NQ), np.float32)
    for j in range(4):
        m[:, j, :] = ((KT * j + k) <= q).astype(np.float32)
    return m.astype(ml_dtypes.bfloat16)


_STATE = {}


def _prep_inputs(x, Wq, bq, Wk, bk, Wv, bv, Wo, bo):
    bf = ml_dtypes.bfloat16
    xT = np.ascontiguousarray(np.asarray(x, np.float32).reshape(S, D).T).astype(bf)
    masks = _make_masks()
    Wq = np.asarray(Wq, np.float32)
    Wk = np.asarray(Wk, np.float32)
    Wv = np.asarray(Wv, np.float32)
    Wo = np.asarray(Wo, np.float32)
    bq = np.asarray(bq, np.float32)
    bk = np.asarray(bk, np.float32)
    bv = np.asarray(bv, np.float32)
    in_maps = []
    for c in range(N_CORES):
        r = slice(c * P, (c + 1) * P)
        in_maps.append({
            "xT": xT,
            "wqT": np.ascontiguousarray(Wq[r].T).astype(bf),
            "wkT": np.ascontiguousarray(Wk[r].T).astype(bf),
            "wvT": np.ascontiguousarray(Wv[r].T).astype(bf),
            "woT": np.ascontiguousarray(Wo[:, r].T).astype(bf),
            "bqk": np.stack([bq[r], bk[r], bv[r]]),
            "masks": masks,
        })
    return in_maps


def kernel(x, Wq, bq, Wk, bk, Wv, bv, Wo, bo):
    if "nc" not in _STATE:
        _STATE["nc"] = build()
    nc = _STATE["nc"]
    in_maps = _prep_inputs(x, Wq, bq, Wk, bk, Wv, bv, Wo, bo)
    res = run_bass_kernel_spmd(nc, in_maps, core_ids=list(range(N_CORES)))
    total = res.results[0]["outT"].astype(np.float32, copy=True)
    for c in range(1, N_CORES):
        total += res.results[c]["outT"]
    out = total.T + np.asarray(bo, np.float32)[None, :]
    return np.ascontiguousarray(out, dtype=np.float32).reshape(1, S, D)
